# revision 12
# baseline (speedup 1.0000x reference)
"""Trainium2 Bass kernel for nn_AlgorithmAMultinomial: top-32 of
log(rand)/probs per row (weighted sampling without replacement), batch
sharded over 8 NeuronCores. See build_nc docstring below for the algorithm.
"""

"""Bass/Tile kernel: weighted sampling without replacement (exponential race).

Per core: probs/rand [128, 128000] f32 -> top-32 indices [128, 32] (uint32 in
DRAM, reinterpreted as int32 on host).

The stream is bound by four engines that all sit near the per-NC DMA rate,
so the per-element transform is split across two order-equivalent schemes
to balance engine load:

  scheme-1: g = ln(p) - ln(-ln u)       ACT x2, GPSIMD subtract
  scheme-2: g = ln(u) * recip~(p)       ACT x1, DVE recip + mult

Each segment's u and p halves live in ONE SBUF tile [R, 2*sz], so scheme-1
computes ln over both halves with a single ACTIVATE (pass 2 then re-lns the
u half in place, scale=-1). recip~ is the single-instruction DVE
RECIPROCAL_APPROX_FAST (~51 ULP); the rank-32 boundary gap of this key
distribution is >=1.6e-4 relative, so a 3e-6 perturbation only risks
intra-top-32 order swaps in a handful of rows. Scheme-2 runs entirely on
ACT+DVE (no cross-engine reciprocal handoff), its segments are sized so
the DVE's 4-pass block matches the pipeline cadence, and the engine totals
stay at or below the proven-safe levels of the 3-pass baseline (the
device's DVFS governor drops 1.2 -> 1.0 GHz when pushed harder).

Stage 1: per segment, top-8 values (DVE max) + local indices (DVE
         max_index) -> candidates V1 [128, W] f32, L [128, W] u32.
         scheme-1 segs own the low candidate groups, scheme-2 the high
         ones, so the scheme-2 block is contiguous and is converted to the
         scheme-1 (log) scale in the tail with one DVE recip + one small
         ACT pass: v = -ln(-v2) = ln(-recip(v2)). Only the natural_log
         table set is ever used -> one ACT_TABLE_LOAD total.
Stage 2: 4 rounds of max/max_index/match_replace on V1 -> top-32 candidate
         slots `pos` in descending value order.
Gather:  G[slot] = seg_base + local (global index, < 2^24 so exact in the
         fp32 DVE ALU); then per output column k
             out[k] = sum((SLOT == pos_k) * G)
         via one scalar_tensor_tensor (op0=is_equal, op1=mult, accum=sum)
         with pos_k as a per-partition fp32 scalar operand. The elementwise
         outputs ping-pong across scratch tiles so consecutive ops don't
         serialize on the accumulator drain.
"""

from contextlib import ExitStack

import concourse.bacc as bacc
import concourse.mybir as mybir
import concourse.tile as tile

R = 128          # rows per core (batch 1024 / 8 cores)
V = 128000       # vocab
# Segment schedule: short ramp, 5300-wide scheme-1 mids interleaved with
# 2000-wide scheme-2 segs (one per three mids), steep taper at the end
# (the tail is a serial ACT->GPSIMD->DVE chain on the last segment).
_MID = []
_MID_SCHEME2 = []
for _k in range(7):
    _n1 = 3 if _k < 6 else 2
    _MID += [5300] * _n1
    _MID_SCHEME2.append(3 + len(_MID))
    _MID += [2000]
SEGS = [1000, 1000, 2000] + _MID + [2000, 1000, 500, 250, 250]
SCHEME2 = tuple(_MID_SCHEME2)
assert sum(SEGS) == V, sum(SEGS)
NSEG = len(SEGS)  # 35
N2 = len(SCHEME2)  # 7
W = NSEG * 8       # 280 candidates per row
W1 = (NSEG - N2) * 8  # first scheme-1 slot block; scheme-2 above it
K = 32
NEG = -3.0e38

F32 = mybir.dt.float32
U32 = mybir.dt.uint32
Ln = mybir.ActivationFunctionType.Ln
Alu = mybir.AluOpType

# slot group (of 8 candidates) per segment: scheme-1 segs in stream order
# get the low groups, scheme-2 segs the high ones.
GROUP = {}
_g1 = 0
_g2 = NSEG - N2
for _ci in range(NSEG):
    if _ci in SCHEME2:
        GROUP[_ci] = _g2
        _g2 += 1
    else:
        GROUP[_ci] = _g1
        _g1 += 1
BASE = {}
_off = 0
for _ci, _sz in enumerate(SEGS):
    BASE[_ci] = _off
    _off += _sz


def _affine_runs():
    """Maximal runs of consecutive slot groups with a constant base step,
    for SEGB iota emission: list of (first_group, ngroups, base0, step)."""
    base_of_group = [0] * NSEG
    for ci in range(NSEG):
        base_of_group[GROUP[ci]] = BASE[ci]
    runs = []
    g = 0
    while g < NSEG:
        b0 = base_of_group[g]
        if g + 1 == NSEG:
            runs.append((g, 1, b0, 0))
            g += 1
            continue
        step = base_of_group[g + 1] - b0
        if not -32768 <= step <= 32767:
            runs.append((g, 1, b0, 0))
            g += 1
            continue
        n = 2
        while g + n < NSEG and base_of_group[g + n] - base_of_group[g + n - 1] == step:
            n += 1
        runs.append((g, n, b0, step))
        g += n
    return runs


def build_nc(num_swdge_queues: int = 4):
    nc = bacc.Bacc("TRN2", num_devices=8, num_swdge_queues=num_swdge_queues)
    probs = nc.dram_tensor("probs", [R, V], F32, kind="ExternalInput").ap()
    rand = nc.dram_tensor("rand", [R, V], F32, kind="ExternalInput").ap()
    out = nc.dram_tensor("out_idx", [R, K], U32, kind="ExternalOutput").ap()

    with ExitStack() as ctx:
        tc = ctx.enter_context(tile.TileContext(nc))
        ioup = ctx.enter_context(tc.tile_pool(name="ioup", bufs=3))
        iog = ctx.enter_context(tc.tile_pool(name="iog", bufs=3))
        cand = ctx.enter_context(tc.tile_pool(name="cand", bufs=1))
        small = ctx.enter_context(tc.tile_pool(name="small", bufs=1))

        V1 = cand.tile([R, W], F32, tag="V1")
        L = cand.tile([R, W], U32, tag="L")
        SLOT = cand.tile([R, W], U32, tag="SLOT")
        SEGB = cand.tile([R, W], U32, tag="SEGB")
        DS = [cand.tile([R, W], F32, tag=f"D{i}", name=f"D{i}") for i in range(2)]
        conv = cand.tile([R, N2 * 8], F32, tag="conv")

        # SLOT[j] = j; SEGB[j] = base offset of slot j's segment (iota per
        # affine run of group bases; steps fit int16)
        nc.gpsimd.iota(
            SLOT[:],
            pattern=[[8, NSEG], [1, 8]],
            base=0,
            channel_multiplier=0,
        )
        for g0, ng, b0, step in _affine_runs():
            assert -32768 <= step <= 32767, step
            nc.gpsimd.iota(
                SEGB[:, g0 * 8:(g0 + ng) * 8],
                pattern=[[step, ng], [0, 8]],
                base=b0,
                channel_multiplier=0,
            )

        for ci, sz in enumerate(SEGS):
            up = ioup.tile([R, 2 * sz], F32, tag="up")
            g = iog.tile([R, sz], F32, tag="g")
            nc.sync.dma_start(up[:, 0:sz], rand[:, BASE[ci]:BASE[ci] + sz])
            nc.sync.dma_start(up[:, sz:2 * sz], probs[:, BASE[ci]:BASE[ci] + sz])
            if ci in SCHEME2:
                # t = ln(u) on the u half; g = t * recip~(p), all DVE
                nc.scalar.activation(up[:, 0:sz], up[:, 0:sz], Ln)
                nc.vector.reciprocal_approx_fast(g[:], up[:, sz:2 * sz])
                nc.vector.tensor_tensor(g[:], up[:, 0:sz], g[:], Alu.mult)
            else:
                # one ACTIVATE lns both halves; u half re-lnd in place
                nc.scalar.activation(up[:], up[:], Ln)
                nc.scalar.activation(up[:, 0:sz], up[:, 0:sz], Ln, scale=-1.0)
                nc.gpsimd.tensor_tensor(
                    g[:], up[:, sz:2 * sz], up[:, 0:sz], Alu.subtract
                )
            j0 = GROUP[ci] * 8
            nc.vector.max(V1[:, j0:j0 + 8], g[:])
            nc.vector.max_index(L[:, j0:j0 + 8], V1[:, j0:j0 + 8], g[:])

        # Convert the scheme-2 candidate block to the scheme-1 (log) scale:
        # v = -ln(-v2) = ln(-recip(v2)).
        nc.vector.reciprocal_approx_fast(conv[:], V1[:, W1:W])
        nc.scalar.activation(V1[:, W1:W], conv[:], Ln, scale=-1.0)

        # G = seg_base + local  (< 2^24, exact in the fp32 DVE ALU); in place.
        nc.vector.tensor_tensor(L[:], L[:], SEGB[:], Alu.add)

        m8 = small.tile([R, 8], F32, tag="m8")
        pos = small.tile([R, K], U32, tag="pos")
        posf = small.tile([R, K], F32, tag="posf")
        outp = small.tile([R, K], F32, tag="outp")
        outi = small.tile([R, K], U32, tag="outi")

        for rnd in range(4):
            nc.vector.max(m8[:], V1[:])
            nc.vector.max_index(pos[:, rnd * 8:(rnd + 1) * 8], m8[:], V1[:])
            if rnd < 3:
                nc.vector.match_replace(V1[:], m8[:], V1[:], NEG)

        # scalar operands are read as fp32 on the DVE
        nc.vector.tensor_copy(posf[:], pos[:])

        for k in range(K):
            nc.vector.scalar_tensor_tensor(
                out=DS[k % 2][:],
                in0=SLOT[:],
                scalar=posf[:, k:k + 1],
                in1=L[:],
                op0=Alu.is_equal,
                op1=Alu.mult,
                accum_out=outp[:, k:k + 1],
            )

        nc.vector.tensor_copy(outi[:], outp[:])
        nc.sync.dma_start(out[:, :], outi[:])

    nc.compile()
    return nc


import numpy as np
from concourse.bass_utils import run_bass_kernel_spmd

N_CORES = 8
B = 1024


_NC_CACHE = None


def _get_nc():
    global _NC_CACHE
    if _NC_CACHE is None:
        _NC_CACHE = build_nc()
    return _NC_CACHE


def run(probs: np.ndarray, rand: np.ndarray, trace: bool = False):
    """Run on 8 NeuronCores; returns (out [1024,32] int32, BassKernelResults)."""
    probs = np.ascontiguousarray(probs, dtype=np.float32)
    rand = np.ascontiguousarray(rand, dtype=np.float32)
    assert probs.shape == (B, V) and rand.shape == (B, V)
    in_maps = [
        {"probs": probs[i * R:(i + 1) * R], "rand": rand[i * R:(i + 1) * R]}
        for i in range(N_CORES)
    ]
    res = run_bass_kernel_spmd(
        _get_nc(), in_maps, core_ids=list(range(N_CORES)), trace=trace
    )
    out = np.concatenate(
        [res.results[i]["out_idx"].astype(np.int32) for i in range(N_CORES)], axis=0
    )
    return out, res


def kernel(probs: np.ndarray, rand: np.ndarray) -> np.ndarray:
    out, _ = run(probs, rand, trace=False)
    return out


# revision 16
# speedup vs baseline: 1.3216x; 1.3216x over previous
"""Trainium2 Bass kernel for nn_AlgorithmAMultinomial: top-32 of
log(rand)/probs per row (weighted sampling without replacement), batch
sharded over 8 NeuronCores. See build_nc docstring below for the algorithm.
"""

"""Bass/Tile kernel: weighted sampling without replacement (exponential race).

Per core: probs/rand [128, 128000] f32 -> top-32 indices [128, 32] (uint32 in
DRAM, reinterpreted as int32 on host).

The stream is bound by four engines that all sit near the per-NC DMA rate,
so the per-element transform is split across two order-equivalent schemes
to balance engine load:

  scheme-1: g = ln(p) - ln(-ln u)       ACT x2, GPSIMD subtract
  scheme-2: g = ln(u) * recip~(p)       ACT x1, DVE recip + mult

Each segment's u and p halves live in ONE SBUF tile [R, 2*sz], so scheme-1
computes ln over both halves with a single ACTIVATE (pass 2 then re-lns the
u half in place, scale=-1). recip~ is the single-instruction DVE
RECIPROCAL_APPROX_FAST (~51 ULP); the rank-32 boundary gap of this key
distribution is >=1.6e-4 relative, so a 3e-6 perturbation only risks
intra-top-32 order swaps in a handful of rows. Scheme-2 runs entirely on
ACT+DVE (no cross-engine reciprocal handoff), its segments are sized so
the DVE's 4-pass block matches the pipeline cadence, and the engine totals
stay at or below the proven-safe levels of the 3-pass baseline (the
device's DVFS governor drops 1.2 -> 1.0 GHz when pushed harder).

Stage 1: per segment, top-8 values (DVE max) + local indices (DVE
         max_index) -> candidates V1 [128, W] f32, L [128, W] u32.
         scheme-1 segs own the low candidate groups, scheme-2 the high
         ones, so the scheme-2 block is contiguous and is converted to the
         scheme-1 (log) scale in the tail with one DVE recip + one small
         ACT pass: v = -ln(-v2) = ln(-recip(v2)). Only the natural_log
         table set is ever used -> one ACT_TABLE_LOAD total.
Stage 2: 4 rounds of max/max_index/match_replace on V1 -> top-32 candidate
         slots `pos` in descending value order.
Gather:  G[slot] = seg_base + local (global index, < 2^24 so exact in the
         fp32 DVE ALU); then per output column k
             out[k] = sum((SLOT == pos_k) * G)
         via one scalar_tensor_tensor (op0=is_equal, op1=mult, accum=sum)
         with pos_k as a per-partition fp32 scalar operand. The elementwise
         outputs ping-pong across scratch tiles so consecutive ops don't
         serialize on the accumulator drain.
"""

from contextlib import ExitStack

import concourse.bacc as bacc
import concourse.mybir as mybir
import concourse.tile as tile

R = 128          # rows per core (batch 1024 / 8 cores)
V = 128000       # vocab
# Segment schedule: short ramp, 5300-wide scheme-1 mids interleaved with
# 2000-wide scheme-2 segs (one per three mids), steep taper at the end
# (the tail is a serial ACT->GPSIMD->DVE chain on the last segment).
SEGS = [1000, 1000, 2000] + [4000] * 30 + [2000, 1000, 500, 250, 250]
SCHEME2 = ()
assert sum(SEGS) == V, sum(SEGS)
NSEG = len(SEGS)  # 38
N2 = len(SCHEME2)  # 0
W = NSEG * 8       # 304 candidates per row
W1 = (NSEG - N2) * 8
K = 32
NEG = -3.0e38

F32 = mybir.dt.float32
U32 = mybir.dt.uint32
Ln = mybir.ActivationFunctionType.Ln
Alu = mybir.AluOpType

# slot group (of 8 candidates) per segment: scheme-1 segs in stream order
# get the low groups, scheme-2 segs the high ones.
GROUP = {}
_g1 = 0
_g2 = NSEG - N2
for _ci in range(NSEG):
    if _ci in SCHEME2:
        GROUP[_ci] = _g2
        _g2 += 1
    else:
        GROUP[_ci] = _g1
        _g1 += 1
BASE = {}
_off = 0
for _ci, _sz in enumerate(SEGS):
    BASE[_ci] = _off
    _off += _sz


def _affine_runs():
    """Maximal runs of consecutive slot groups with a constant base step,
    for SEGB iota emission: list of (first_group, ngroups, base0, step)."""
    base_of_group = [0] * NSEG
    for ci in range(NSEG):
        base_of_group[GROUP[ci]] = BASE[ci]
    runs = []
    g = 0
    while g < NSEG:
        b0 = base_of_group[g]
        if g + 1 == NSEG:
            runs.append((g, 1, b0, 0))
            g += 1
            continue
        step = base_of_group[g + 1] - b0
        if not -32768 <= step <= 32767:
            runs.append((g, 1, b0, 0))
            g += 1
            continue
        n = 2
        while g + n < NSEG and base_of_group[g + n] - base_of_group[g + n - 1] == step:
            n += 1
        runs.append((g, n, b0, step))
        g += n
    return runs


def build_nc(num_swdge_queues: int = 4):
    nc = bacc.Bacc("TRN2", num_devices=8, num_swdge_queues=num_swdge_queues)
    probs = nc.dram_tensor("probs", [R, V], F32, kind="ExternalInput").ap()
    rand = nc.dram_tensor("rand", [R, V], F32, kind="ExternalInput").ap()
    out = nc.dram_tensor("out_idx", [R, K], U32, kind="ExternalOutput").ap()

    with ExitStack() as ctx:
        tc = ctx.enter_context(tile.TileContext(nc))
        ioup = ctx.enter_context(tc.tile_pool(name="ioup", bufs=4))
        iog = ctx.enter_context(tc.tile_pool(name="iog", bufs=3))
        cand = ctx.enter_context(tc.tile_pool(name="cand", bufs=1))
        small = ctx.enter_context(tc.tile_pool(name="small", bufs=1))

        V1 = cand.tile([R, W], F32, tag="V1")
        L = cand.tile([R, W], U32, tag="L")
        SLOT = cand.tile([R, W], U32, tag="SLOT")
        SEGB = cand.tile([R, W], U32, tag="SEGB")
        DS = [cand.tile([R, W], F32, tag=f"D{i}", name=f"D{i}") for i in range(2)]
        conv = cand.tile([R, N2 * 8], F32, tag="conv") if N2 else None

        # SLOT[j] = j; SEGB[j] = base offset of slot j's segment (iota per
        # affine run of group bases; steps fit int16)
        nc.gpsimd.iota(
            SLOT[:],
            pattern=[[8, NSEG], [1, 8]],
            base=0,
            channel_multiplier=0,
        )
        for g0, ng, b0, step in _affine_runs():
            assert -32768 <= step <= 32767, step
            nc.gpsimd.iota(
                SEGB[:, g0 * 8:(g0 + ng) * 8],
                pattern=[[step, ng], [0, 8]],
                base=b0,
                channel_multiplier=0,
            )

        for ci, sz in enumerate(SEGS):
            up = ioup.tile([R, 2 * sz], F32, tag="up")
            g = iog.tile([R, sz], F32, tag="g")
            nc.sync.dma_start(up[:, 0:sz], rand[:, BASE[ci]:BASE[ci] + sz])
            nc.sync.dma_start(up[:, sz:2 * sz], probs[:, BASE[ci]:BASE[ci] + sz])
            if ci in SCHEME2:
                # t = ln(u) on the u half; g = t * recip~(p), all DVE
                nc.scalar.activation(up[:, 0:sz], up[:, 0:sz], Ln)
                nc.vector.reciprocal_approx_fast(g[:], up[:, sz:2 * sz])
                nc.vector.tensor_tensor(g[:], up[:, 0:sz], g[:], Alu.mult)
            else:
                # one ACTIVATE lns both halves; u half re-lnd in place
                nc.scalar.activation(up[:], up[:], Ln)
                nc.scalar.activation(up[:, 0:sz], up[:, 0:sz], Ln, scale=-1.0)
                nc.gpsimd.tensor_tensor(
                    g[:], up[:, sz:2 * sz], up[:, 0:sz], Alu.subtract
                )
            j0 = GROUP[ci] * 8
            nc.vector.max(V1[:, j0:j0 + 8], g[:])
            nc.vector.max_index(L[:, j0:j0 + 8], V1[:, j0:j0 + 8], g[:])

        if N2:
            # Convert the scheme-2 candidate block to the scheme-1 (log)
            # scale: v = -ln(-v2) = ln(-recip(v2)).
            nc.vector.reciprocal_approx_fast(conv[:], V1[:, W1:W])
            nc.scalar.activation(V1[:, W1:W], conv[:], Ln, scale=-1.0)

        # G = seg_base + local  (< 2^24, exact in the fp32 DVE ALU); in place.
        nc.vector.tensor_tensor(L[:], L[:], SEGB[:], Alu.add)

        m8 = small.tile([R, 8], F32, tag="m8")
        pos = small.tile([R, K], U32, tag="pos")
        posf = small.tile([R, K], F32, tag="posf")
        outp = small.tile([R, K], F32, tag="outp")
        outi = small.tile([R, K], U32, tag="outi")

        for rnd in range(4):
            nc.vector.max(m8[:], V1[:])
            nc.vector.max_index(pos[:, rnd * 8:(rnd + 1) * 8], m8[:], V1[:])
            if rnd < 3:
                nc.vector.match_replace(V1[:], m8[:], V1[:], NEG)

        # scalar operands are read as fp32 on the DVE
        nc.vector.tensor_copy(posf[:], pos[:])

        for k in range(K):
            nc.vector.scalar_tensor_tensor(
                out=DS[k % 2][:],
                in0=SLOT[:],
                scalar=posf[:, k:k + 1],
                in1=L[:],
                op0=Alu.is_equal,
                op1=Alu.mult,
                accum_out=outp[:, k:k + 1],
            )

        nc.vector.tensor_copy(outi[:], outp[:])
        nc.sync.dma_start(out[:, :], outi[:])

    nc.compile()
    return nc


import numpy as np
from concourse.bass_utils import run_bass_kernel_spmd

N_CORES = 8
B = 1024


_NC_CACHE = None


def _get_nc():
    global _NC_CACHE
    if _NC_CACHE is None:
        _NC_CACHE = build_nc()
    return _NC_CACHE


def run(probs: np.ndarray, rand: np.ndarray, trace: bool = False):
    """Run on 8 NeuronCores; returns (out [1024,32] int32, BassKernelResults)."""
    probs = np.ascontiguousarray(probs, dtype=np.float32)
    rand = np.ascontiguousarray(rand, dtype=np.float32)
    assert probs.shape == (B, V) and rand.shape == (B, V)
    in_maps = [
        {"probs": probs[i * R:(i + 1) * R], "rand": rand[i * R:(i + 1) * R]}
        for i in range(N_CORES)
    ]
    res = run_bass_kernel_spmd(
        _get_nc(), in_maps, core_ids=list(range(N_CORES)), trace=trace
    )
    out = np.concatenate(
        [res.results[i]["out_idx"].astype(np.int32) for i in range(N_CORES)], axis=0
    )
    return out, res


def kernel(probs: np.ndarray, rand: np.ndarray) -> np.ndarray:
    out, _ = run(probs, rand, trace=False)
    return out
